# revision 9
# baseline (speedup 1.0000x reference)
import sys
sys.path.insert(0, '/opt/trn_rl_repo')
import numpy as np
import concourse.bass as bass
import concourse.bacc as bacc
import concourse.tile as tile
import concourse.mybir as mybir
import bass_rust
from concourse.bass_utils import run_bass_kernel_spmd

F32 = mybir.dt.float32
F16 = mybir.dt.float16
AF = mybir.ActivationFunctionType
ALU = mybir.AluOpType

NCORES = 8
CIN, COUT = 32, 64
D1, D2, D3 = 160, 160, 32
OWN = D1 // NCORES          # 20 owned d1-rows per core
D2P, D3P = D2 + 2, D3 + 1   # padded plane: 162 x 33 (d3 pad col shared)
PLANE = D2P * D3P           # 5346
G = 34                      # tile guard columns each side (max |shift| = 34)
W = PLANE + 2 * G + 2       # sbuf row-tile width
NSPLIT = 15 * D3P           # matmul chunk: 15 d2-rows = 495 positions
EPS = 1e-5
SLOPE = 0.01

# pool output geometry: 10 out rows/core, positions (do2 in [0,80), do3 in [0,32))
PO_ROWS = OWN // 2
PO_N = 80 * 32              # 2560, chunked by 16 do2-rows = 512
PO_CHUNK = 16 * 32

XROWS = 23   # x / mask rows per core: logical d1 = own_start-2 .. own_start+20
ZA1_ROWS = 21  # rows -1..19 (slot = r+1)
ZB1_ROWS = 23  # rows -2..20 (slot = r+2)
Z2_ROWS = 21   # zA2/zB2 rows -1..19 (slot = r+1)


def _win(ap, dims):
    """Overlapping multi-dim window view: keep ap's partition dim + offset,
    replace free dims with [[step, count], ...] (element units)."""
    c = ap.copy()
    part = [list(p) for p in c.ap][0]
    c.ap = bass_rust.VecI64Pair([part] + [list(d) for d in dims])
    return c


def _chunks():
    """(start, size) chunks of the plane, d2-aligned, size<=512."""
    out = []
    s = 0
    while s < PLANE:
        n = min(NSPLIT, PLANE - s)
        out.append((s, n))
        s += n
    return out


CHUNKS = _chunks()  # 10x495 + 396

TRACE = False          # set by test.py to capture an NTFF profile
LAST_RESULTS = None
_NC_CACHE = None


def build_program():
    global _NC_CACHE
    if _NC_CACHE is not None:
        return _NC_CACHE
    nc = bacc.Bacc("TRN2", target_bir_lowering=False, debug=False,
                   num_devices=NCORES)

    # ---- external I/O (per-core shards) ----
    x_sh = nc.dram_tensor("x_sh", [CIN, XROWS, PLANE], F16, kind="ExternalInput")
    m_sh = nc.dram_tensor("m_sh", [COUT, XROWS, PLANE], F16, kind="ExternalInput")
    mflat = nc.dram_tensor("mflat", [128, 800], F32, kind="ExternalInput")
    wA1 = nc.dram_tensor("wA1", [96, 3, 64], F16, kind="ExternalInput")
    wB1 = nc.dram_tensor("wB1", [96, 3, 64], F16, kind="ExternalInput")
    wA2f = nc.dram_tensor("wA2f", [128, 3, 64], F16, kind="ExternalInput")
    wA2h = nc.dram_tensor("wA2h", [64, 3, 64], F16, kind="ExternalInput")
    wB2f = nc.dram_tensor("wB2f", [128, 3, 64], F16, kind="ExternalInput")
    wB2h = nc.dram_tensor("wB2h", [64, 3, 64], F16, kind="ExternalInput")
    wPf = nc.dram_tensor("wPf", [128, 9, 64], F16, kind="ExternalInput")
    wPh = nc.dram_tensor("wPh", [64, 9, 64], F16, kind="ExternalInput")
    gb = nc.dram_tensor("gb", [64, 8], F32, kind="ExternalInput")
    res_out = nc.dram_tensor("res_out", [COUT, OWN, PLANE], F16,
                             kind="ExternalOutput")
    down_out = nc.dram_tensor("down_out", [COUT, PO_ROWS, PO_N], F32,
                              kind="ExternalOutput")

    with tile.TileContext(nc) as tc:
        with (
            tc.tile_pool(name="wpool", bufs=1) as wpool,
            tc.tile_pool(name="stats", bufs=1) as spool,
            tc.tile_pool(name="dram", bufs=1, space="DRAM") as dram,
            tc.tile_pool(name="psum_conv", bufs=2, space="PSUM") as pconv,
            tc.tile_pool(name="psum_aux", bufs=2, space="PSUM") as paux,
        ):
            # ---------- persistent weights ----------
            wA1t = wpool.tile([96, 3, 64], F16)
            wB1t = wpool.tile([96, 3, 64], F16)
            wA2ft = wpool.tile([128, 3, 64], F16)
            wA2ht = wpool.tile([64, 3, 64], F16)
            wB2ft = wpool.tile([128, 3, 64], F16)
            wB2ht = wpool.tile([64, 3, 64], F16)
            wPft = wpool.tile([128, 9, 64], F16)
            wPht = wpool.tile([64, 9, 64], F16)
            # BN-scaled copies for pass 2
            wA2fs = wpool.tile([128, 3, 64], F16)
            wA2hs = wpool.tile([64, 3, 64], F16)
            wB2fs = wpool.tile([128, 3, 64], F16)
            wB2hs = wpool.tile([64, 3, 64], F16)
            gbt = wpool.tile([64, 8], F32)
            for t, d in ((wA1t, wA1), (wB1t, wB1), (wA2ft, wA2f), (wA2ht, wA2h),
                         (wB2ft, wB2f), (wB2ht, wB2h), (wPft, wPf), (wPht, wPh),
                         (gbt, gb)):
                nc.sync.dma_start(t[:], d[:])

            # ---------- n_active (replicated on 64 partitions) ----------
            mft = spool.tile([128, 800], F32)
            nc.sync.dma_start(mft[:], mflat[:])
            mred = spool.tile([128, 1], F32)
            nc.vector.tensor_reduce(mred[:], mft[:], axis=mybir.AxisListType.X,
                                    op=ALU.add)
            ones128 = spool.tile([128, 64], F16)
            nc.vector.memset(ones128[:], 1.0)
            mred16 = spool.tile([128, 1], F16)
            nc.vector.tensor_copy(mred16[:], mred[:])
            nps = paux.tile([64, 1], F32)
            nc.tensor.matmul(nps[:], ones128[:], mred16[:], start=True, stop=True)
            nvec = spool.tile([64, 1], F32)
            nc.vector.tensor_copy(nvec[:], nps[:])

            # dram intermediates
            zA1d = dram.tile([COUT, ZA1_ROWS, PLANE], F16)
            zB1d = dram.tile([COUT, ZB1_ROWS, PLANE], F16)
            zA2d = dram.tile([COUT, Z2_ROWS, PLANE], F16)
            zB2d = dram.tile([COUT, Z2_ROWS, PLANE], F16)

            # stats accumulators (per conv layer): per-row (mean, var)
            rowagg = {k: spool.tile([64, OWN, 2], F32, name=f"rowagg_{k}")
                      for k in ("A1", "B1", "A2", "B2")}

            # ======================================================
            # PASS 1:  A1 = conv(xs, W_A1 (3,1,3)),  B1 = conv(xs, W_B1 (1,3,3))
            # ======================================================
            with (
                tc.tile_pool(name="pk1", bufs=4) as pk1pool,
                tc.tile_pool(name="mrow1", bufs=4) as mpool1,
                tc.tile_pool(name="zst1", bufs=2) as zst1,
                tc.tile_pool(name="tmp1", bufs=3) as tmp1,
                tc.tile_pool(name="bst1", bufs=2) as bst1,
            ):
                packs = {}   # xr -> pack tile [96, W]
                mrows = {}   # xr -> mask row tile [64, W]

                def load_row_p1(xr):
                    pk = pk1pool.tile([96, W], F16, tag="pk")
                    mr = mpool1.tile([64, W], F16, tag="mr")
                    if xr < 4:
                        nc.vector.memset(pk[:], 0.0)
                        nc.vector.memset(mr[:], 0.0)
                    nc.sync.dma_start(mr[:, G:G + PLANE], m_sh[:, xr, :])
                    nc.sync.dma_start(pk[32:64, G:G + PLANE], x_sh[:, xr, :])
                    # mask in place:  xs = x * m
                    nc.vector.tensor_tensor(
                        out=pk[32:64, G:G + PLANE], in0=pk[32:64, G:G + PLANE],
                        in1=mr[32:64, G:G + PLANE], op=ALU.mult)
                    # shifted replicas: block0 = xs(pos-1), block2 = xs(pos+1)
                    nc.sync.dma_start(pk[0:32, 1:W], pk[32:64, 0:W - 1])
                    nc.sync.dma_start(pk[64:96, 0:W - 1], pk[32:64, 1:W])
                    packs[xr] = pk
                    mrows[xr] = mr

                def conv_row(kind, r):
                    # kind 'A1': out-row r, taps (dd, dw): packs r-1,r,r+1
                    # kind 'B1': out-row r, taps (dh, dw): pack r only
                    zrow = zst1.tile([64, PLANE], F16, tag=f"z{kind}")
                    own = 0 <= r < OWN
                    if own:
                        bst = bst1.tile([64, len(CHUNKS), 6], F32, tag=f"b{kind}")
                    mr = mrows[r + 2]
                    for ci, (cs, cn) in enumerate(CHUNKS):
                        ps = pconv.tile([64, NSPLIT], F32, tag="conv")
                        for k in range(3):
                            if kind == "A1":
                                pk = packs[r + 1 + k]
                                rhs = pk[0:96, G + cs:G + cs + cn]
                                lhsT = wA1t[:, k, :]
                            else:
                                pk = packs[r + 2]
                                off = (k - 1) * D3P
                                rhs = pk[0:96, G + cs + off:G + cs + off + cn]
                                lhsT = wB1t[:, k, :]
                            nc.tensor.matmul(ps[:, 0:cn], lhsT, rhs,
                                             start=(k == 0), stop=(k == 2))
                        tchunk = tmp1.tile([64, NSPLIT], F16, tag="t")
                        nc.scalar.activation(tchunk[:, 0:cn], ps[:, 0:cn],
                                             AF.Lrelu, alpha=SLOPE)
                        nc.vector.tensor_tensor(
                            out=zrow[:, cs:cs + cn], in0=tchunk[:, 0:cn],
                            in1=mr[:, G + cs:G + cs + cn], op=ALU.mult)
                        if own:
                            nc.vector.bn_stats(bst[:, ci, :], zrow[:, cs:cs + cn])
                    if own:
                        nc.vector.bn_aggr(rowagg[kind][:, r, :], bst[:])
                    dst = zA1d if kind == "A1" else zB1d
                    slot = r + 1 if kind == "A1" else r + 2
                    nc.sync.dma_start(dst[:, slot, :], zrow[:])

                for xr in range(XROWS):
                    load_row_p1(xr)
                    rl = xr - 2           # logical d1 row just loaded
                    if -2 <= rl <= 20:
                        conv_row("B1", rl)
                    ra = rl - 1
                    if -1 <= ra < 20:
                        conv_row("A1", ra)
                    # free old pack/mask refs (pool rotation handles reuse)
                    packs.pop(xr - 3, None)
                    mrows.pop(xr - 3, None)

            # ======================================================
            # stats -> allreduce #1 -> BN affine params for A1, B1
            # ======================================================
            def finalize_stats(keys, tag):
                st = spool.tile([64, 5], F32, name=f"stats_{tag}")
                for i, k in enumerate(keys):
                    ra = rowagg[k]
                    t1 = spool.tile([64, OWN], F32, name=f"t1_{k}")
                    nc.vector.tensor_tensor(out=t1[:], in0=ra[:, :, 0],
                                            in1=ra[:, :, 0], op=ALU.mult)
                    nc.vector.tensor_tensor(out=t1[:], in0=t1[:],
                                            in1=ra[:, :, 1], op=ALU.add)
                    # sum z = PLANE * sum(mean_r);  sum z^2 = PLANE * sum(var+mean^2)
                    s0 = spool.tile([64, 1], F32, name=f"s0_{k}")
                    nc.vector.tensor_reduce(s0[:], ra[:, :, 0],
                                            axis=mybir.AxisListType.X, op=ALU.add)
                    nc.vector.tensor_scalar_mul(st[:, 2 * i:2 * i + 1], s0[:],
                                                float(PLANE))
                    s1 = spool.tile([64, 1], F32, name=f"s1_{k}")
                    nc.vector.tensor_reduce(s1[:], t1[:],
                                            axis=mybir.AxisListType.X, op=ALU.add)
                    nc.vector.tensor_scalar_mul(st[:, 2 * i + 1:2 * i + 2], s1[:],
                                                float(PLANE))
                nc.vector.tensor_copy(st[:, 4:5], nvec[:])
                bin_ = dram.tile([64, 5], F32, name=f"arin_{tag}")
                bout = dram.tile([64, 5], F32, name=f"arout_{tag}")
                nc.sync.dma_start(bin_[:], st[:])
                nc.gpsimd.collective_compute(
                    "AllReduce", ALU.add,
                    replica_groups=[list(range(NCORES))],
                    ins=[bin_.opt()], outs=[bout.opt()])
                stg = spool.tile([64, 5], F32, name=f"arres_{tag}")
                nc.sync.dma_start(stg[:], bout[:])
                return stg

            def bn_params(stg, i, gcol, bcol, tag):
                # returns (a [64,1] f32, b [64,1] f32)
                rn = spool.tile([64, 1], F32, name=f"rn_{tag}")
                nc.vector.reciprocal(rn[:], stg[:, 4:5])
                mu = spool.tile([64, 1], F32, name=f"mu_{tag}")
                nc.vector.tensor_tensor(out=mu[:], in0=stg[:, 2 * i:2 * i + 1],
                                        in1=rn[:], op=ALU.mult)
                ez2 = spool.tile([64, 1], F32, name=f"ez2_{tag}")
                nc.vector.tensor_tensor(out=ez2[:], in0=stg[:, 2 * i + 1:2 * i + 2],
                                        in1=rn[:], op=ALU.mult)
                var = spool.tile([64, 1], F32, name=f"var_{tag}")
                nc.vector.tensor_tensor(out=var[:], in0=mu[:], in1=mu[:],
                                        op=ALU.mult)
                nc.vector.tensor_tensor(out=var[:], in0=ez2[:], in1=var[:],
                                        op=ALU.subtract)
                nc.vector.tensor_scalar_add(var[:], var[:], EPS)
                sd = spool.tile([64, 1], F32, name=f"sd_{tag}")
                nc.scalar.activation(sd[:], var[:], AF.Sqrt)
                inv = spool.tile([64, 1], F32, name=f"inv_{tag}")
                nc.vector.reciprocal(inv[:], sd[:])
                a = spool.tile([64, 1], F32, name=f"a_{tag}")
                nc.vector.tensor_tensor(out=a[:], in0=inv[:],
                                        in1=gbt[:, gcol:gcol + 1], op=ALU.mult)
                b = spool.tile([64, 1], F32, name=f"b_{tag}")
                nc.vector.tensor_tensor(out=b[:], in0=mu[:], in1=a[:],
                                        op=ALU.mult)
                nc.vector.tensor_tensor(out=b[:], in0=gbt[:, bcol:bcol + 1],
                                        in1=b[:], op=ALU.subtract)
                return a, b

            def scale_weights(a, full_raw, full_s, half_raw, half_s, tag):
                a128 = spool.tile([128, 1], F32, name=f"a128_{tag}")
                nc.vector.tensor_copy(a128[0:64, :], a[:])
                nc.sync.dma_start(a128[64:128, :], a[:])
                nc.vector.tensor_scalar_mul(
                    full_s[:].rearrange("p a b -> p (a b)"),
                    full_raw[:].rearrange("p a b -> p (a b)"), a128[:])
                nc.vector.tensor_scalar_mul(
                    half_s[:].rearrange("p a b -> p (a b)"),
                    half_raw[:].rearrange("p a b -> p (a b)"), a[:])

            def kappa(b, full_raw, half_raw, order, tag, base=0):
                # kappa[t,o] = sum_i W_raw[o,i,tap]*b_i, laid out [9,64] rows=t
                b16 = spool.tile([128, 1], F16, name=f"b16_{tag}")
                nc.vector.tensor_copy(b16[0:64, :], b[:])
                nc.sync.dma_start(b16[64:128, :], b16[0:64, :])
                kT = spool.tile([64, 32], F32, name=f"kT_{tag}")
                nc.vector.memset(kT[:], 0.0)
                for t, (blk, idx) in enumerate(order):
                    src = full_raw if blk >= 0 else half_raw
                    if blk >= 0:
                        lhsT = src[64 * blk:64 * blk + 64, idx, :]
                        rhs = b16[64 * blk:64 * blk + 64, :]
                    else:
                        lhsT, rhs = src[:, idx, :], b16[0:64, :]
                    kp = paux.tile([64, 1], F32, tag="tiny")
                    nc.tensor.matmul(kp[:], lhsT, rhs, start=True, stop=True)
                    nc.scalar.copy(kT[:, t:t + 1], kp[:])
                kTt = spool.tile([64, 64], F32, name=f"kTt_{tag}")
                nc.vector.transpose(kTt[0:32, 0:32], kT[0:32, :])
                nc.vector.transpose(kTt[0:32, 32:64], kT[32:64, :])
                ka32 = spool.tile([32, 64], F16, name=f"ka32_{tag}")
                nc.vector.tensor_copy(ka32[:], kTt[0:32, :])
                if base == 0:
                    return ka32
                ka = spool.tile([64, 64], F16, name=f"ka_{tag}")
                nc.sync.dma_start(ka[base:base + 9, :], ka32[0:9, :])
                return ka

            stg1 = finalize_stats(("A1", "B1"), "ar1")
            aA1, bA1 = bn_params(stg1, 0, 0, 1, "A1")
            aB1, bB1 = bn_params(stg1, 1, 4, 5, "B1")
            scale_weights(aA1, wA2ft, wA2fs, wA2ht, wA2hs, "A2")
            scale_weights(aB1, wB2ft, wB2fs, wB2ht, wB2hs, "B2")
            # kappa col order must match mp row order:
            # A2 rows: t = kw*3+dh  -> tap (dh, kw)
            ordA2 = [(kw - 1 if kw >= 1 else -1, dh)
                     for kw in range(3) for dh in range(3)]
            # B2 rows: t = dd*3+kw -> tap (dd, kw)
            ordB2 = [(kw - 1 if kw >= 1 else -1, dd)
                     for dd in range(3) for kw in range(3)]
            kaA2 = kappa(bA1, wA2ft, wA2ht, ordA2, "A2")
            kaB2 = kappa(bB1, wB2ft, wB2ht, ordB2, "B2", base=32)

            # ======================================================
            # PASS 2:  A2 = conv(u_A1, (1,3,3)),  B2 = conv(u_B1, (3,1,3))
            #   u = a*z + b*m  folded as: scaled weights + kappa-bias matmul
            # ======================================================
            with (
                tc.tile_pool(name="pkA2", bufs=2) as pkA2pool,
                tc.tile_pool(name="pkB2", bufs=4) as pkB2pool,
                tc.tile_pool(name="mrow2", bufs=4) as mpool2,
                tc.tile_pool(name="mp2", bufs=2) as mppool,
                tc.tile_pool(name="zst2", bufs=2) as zst2,
                tc.tile_pool(name="tmp2", bufs=3) as tmp2,
                tc.tile_pool(name="bst2", bufs=2) as bst2,
            ):
                pkB = {}
                mrows2 = {}

                def load_packA2(r):
                    pk = pkA2pool.tile([128, W], F16, tag="pkA")
                    if r < 1:
                        nc.vector.memset(pk[:], 0.0)
                    nc.sync.dma_start(pk[0:64, G:G + PLANE], zA1d[:, r + 1, :])
                    nc.sync.dma_start(pk[64:128, 0:W - 1], pk[0:64, 1:W])
                    return pk

                def load_packB2(rz):
                    pk = pkB2pool.tile([128, W], F16, tag="pkB")
                    if rz < 2:
                        nc.vector.memset(pk[:], 0.0)
                    nc.sync.dma_start(pk[0:64, G:G + PLANE], zB1d[:, rz + 2, :])
                    nc.sync.dma_start(pk[64:128, 0:W - 1], pk[0:64, 1:W])
                    pkB[rz] = pk

                def load_mask2(r):
                    mr = mpool2.tile([64, W], F16, tag="mr2")
                    if r < 2:
                        nc.vector.memset(mr[:], 0.0)
                    nc.sync.dma_start(mr[:, G:G + PLANE], m_sh[:, r + 2, :])
                    mrows2[r] = mr

                def build_mp(r):
                    # rows 0:9   A2 windows of mask row r: t=kw*3+dh
                    # rows 32:41 B2 windows rows r-1..r+1: t=dd*3+kw
                    # rows 64:67 scratch strip: kw-windows of row r
                    mp = mppool.tile([67, W], F16, tag="mp")
                    mr = mrows2[r]
                    nc.sync.dma_start(
                        mp[64:67, 1:W - 2],
                        _win(mr[0:1, 0:W - 3], [[1, 3], [1, W - 3]]))
                    nc.sync.dma_start(
                        mp[0:9, G:G + PLANE],
                        _win(mp[64:67, G - D3P:G - D3P + PLANE],
                             [[D3P, 3], [1, PLANE]]))
                    for dd in range(3):
                        src = mrows2[r + dd - 1]
                        nc.sync.dma_start(
                            mp[32 + 3 * dd:35 + 3 * dd, G:G + PLANE],
                            _win(src[0:1, G - 1:G - 1 + PLANE],
                                 [[1, 3], [1, PLANE]]))
                    return mp

                def conv_row2(kind, r, pkA, mp):
                    zrow = zst2.tile([64, PLANE], F16, tag=f"z{kind}")
                    own = 0 <= r < OWN
                    if own:
                        bst = bst2.tile([64, len(CHUNKS), 6], F32, tag=f"b{kind}")
                    mr = mrows2[r]
                    for ci, (cs, cn) in enumerate(CHUNKS):
                        ps = pconv.tile([64, NSPLIT], F32, tag="conv")
                        nmm = 0
                        for k in range(3):
                            if kind == "A2":
                                pk, off = pkA, (k - 1) * D3P
                                wf, wh = wA2fs, wA2hs
                            else:
                                pk, off = pkB[r + k - 1], 0
                                wf, wh = wB2fs, wB2hs
                            # full: blocks (j=0,1) = taps kw=1,2 at offset 0
                            rhs = pk[0:128, G + cs + off:G + cs + off + cn]
                            nc.tensor.matmul(ps[:, 0:cn], wf[:, k, :], rhs,
                                             start=(nmm == 0), stop=False)
                            nmm += 1
                            rhs = pk[0:64, G + cs + off - 1:G + cs + off - 1 + cn]
                            nc.tensor.matmul(ps[:, 0:cn], wh[:, k, :], rhs,
                                             start=False, stop=False)
                            nmm += 1
                        if kind == "A2":
                            ka, mpr = kaA2[0:9, :], mp[0:9, G + cs:G + cs + cn]
                        else:
                            ka, mpr = kaB2[32:41, :], mp[32:41, G + cs:G + cs + cn]
                        nc.tensor.matmul(ps[:, 0:cn], ka, mpr,
                                         start=False, stop=True)
                        tchunk = tmp2.tile([64, NSPLIT], F16, tag="t")
                        nc.scalar.activation(tchunk[:, 0:cn], ps[:, 0:cn],
                                             AF.Lrelu, alpha=SLOPE)
                        nc.vector.tensor_tensor(
                            out=zrow[:, cs:cs + cn], in0=tchunk[:, 0:cn],
                            in1=mr[:, G + cs:G + cs + cn], op=ALU.mult)
                        if own:
                            nc.vector.bn_stats(bst[:, ci, :], zrow[:, cs:cs + cn])
                    if own:
                        nc.vector.bn_aggr(rowagg[kind][:, r, :], bst[:])
                    dst = zA2d if kind == "A2" else zB2d
                    nc.sync.dma_start(dst[:, r + 1, :], zrow[:])

                # prologue loads: zB1 rows rz=-2,-1 ; mask rows -2..-1
                for rz in (-2, -1):
                    load_packB2(rz)
                load_mask2(-2)
                load_mask2(-1)
                for r in range(-1, OWN):
                    load_packB2(r + 1)
                    load_mask2(r + 1)
                    pkA = load_packA2(r)
                    mp = build_mp(r)
                    conv_row2("A2", r, pkA, mp)
                    conv_row2("B2", r, pkA, mp)
                    pkB.pop(r - 1, None)
                    mrows2.pop(r - 1, None)

            # ======================================================
            # stats -> allreduce #2 -> res_B materialization + pool conv
            # ======================================================
            stg2 = finalize_stats(("A2", "B2"), "ar2")
            aA2, bA2 = bn_params(stg2, 0, 2, 3, "A2f")
            aB2, bB2 = bn_params(stg2, 1, 6, 7, "B2f")
            # bAB row [1,64] for the rank-1 mask bias matmul
            bAB = spool.tile([64, 1], F32, name="bAB")
            nc.vector.tensor_tensor(out=bAB[:], in0=bA2[:], in1=bB2[:], op=ALU.add)
            bABsq = spool.tile([64, 32], F32, name="bABsq")
            nc.vector.memset(bABsq[:], 0.0)
            nc.vector.tensor_copy(bABsq[:, 0:1], bAB[:])
            bABt = spool.tile([64, 64], F32, name="bABt")
            nc.vector.transpose(bABt[0:32, 0:32], bABsq[0:32, :])
            nc.vector.transpose(bABt[0:32, 32:64], bABsq[32:64, :])
            bABrow = spool.tile([1, 64], F16, name="bABrow")
            nc.vector.tensor_copy(bABrow[:], bABt[0:1, :])

            with (
                tc.tile_pool(name="z3", bufs=2) as z3pool,
                tc.tile_pool(name="res3", bufs=4) as res3pool,
                tc.tile_pool(name="mrow3", bufs=2) as mpool3,
                tc.tile_pool(name="dst3", bufs=2) as dst3,
            ):
                resrows = {}

                def res_row(r):
                    # res = aA2*zA2 + aB2*zB2 + bAB*m   (rows -1..19)
                    zta = z3pool.tile([64, PLANE], F16, tag="z3a")
                    ztb = z3pool.tile([64, PLANE], F16, tag="z3b")
                    nc.sync.dma_start(zta[:], zA2d[:, r + 1, :])
                    nc.sync.dma_start(ztb[:], zB2d[:, r + 1, :])
                    mr = mpool3.tile([64, W], F16, tag="mr3")
                    if r < 1:
                        nc.vector.memset(mr[:], 0.0)
                    nc.sync.dma_start(mr[:, G:G + PLANE], m_sh[:, r + 2, :])
                    pk = res3pool.tile([128, W], F16, tag="res")
                    if r < 3:
                        nc.vector.memset(pk[:], 0.0)
                    for ci, (cs, cn) in enumerate(CHUNKS):
                        cm = paux.tile([64, NSPLIT], F32, tag="cm")
                        nc.tensor.matmul(cm[:, 0:cn], bABrow[:],
                                         mr[0:1, G + cs:G + cs + cn],
                                         start=True, stop=True)
                        t3 = dst3.tile([64, NSPLIT], F32, tag="t3")
                        nc.vector.scalar_tensor_tensor(
                            out=t3[:, 0:cn], in0=zta[:, cs:cs + cn],
                            scalar=aA2[:], in1=cm[:, 0:cn],
                            op0=ALU.mult, op1=ALU.add)
                        nc.vector.scalar_tensor_tensor(
                            out=pk[0:64, G + cs:G + cs + cn],
                            in0=ztb[:, cs:cs + cn],
                            scalar=aB2[:], in1=t3[:, 0:cn],
                            op0=ALU.mult, op1=ALU.add)
                    # replicate for pool pack: block1 = res(pos+1)
                    nc.sync.dma_start(pk[64:128, 0:W - 1], pk[0:64, 1:W])
                    if 0 <= r < OWN:
                        nc.sync.dma_start(res_out[:, r, :],
                                          pk[0:64, G:G + PLANE])
                    resrows[r] = pk

                def pool_row(ro):
                    # down rows: reads res rows 2ro-1, 2ro, 2ro+1
                    for c in range(5):
                        ps = pconv.tile([64, PO_CHUNK], F32, tag="conv")
                        base = c * 16 * 2 * D3P
                        nmm = 0
                        for kd in range(3):
                            pk = resrows[2 * ro + kd - 1]
                            for kh in range(3):
                                off = G + base + kh * D3P
                                rhs = _win(pk[0:128, off + 1:off + 1 + PO_CHUNK],
                                           [[2 * D3P, 16], [1, 32]])
                                nc.tensor.matmul(
                                    ps[:], wPft[:, 3 * kd + kh, :], rhs,
                                    start=(nmm == 0), stop=False)
                                nmm += 1
                                rhs = _win(pk[0:64, off:off + PO_CHUNK],
                                           [[2 * D3P, 16], [1, 32]])
                                last = (kd == 2 and kh == 2)
                                nc.tensor.matmul(
                                    ps[:], wPht[:, 3 * kd + kh, :], rhs,
                                    start=False, stop=last)
                                nmm += 1
                        dc = dst3.tile([64, PO_CHUNK], F32, tag="dchunk")
                        nc.vector.tensor_copy(dc[:], ps[:])
                        nc.sync.dma_start(
                            down_out[:, ro, c * PO_CHUNK:(c + 1) * PO_CHUNK],
                            dc[:])

                res_row(-1)
                res_row(0)
                for ro in range(PO_ROWS):
                    res_row(2 * ro + 1)
                    if 2 * ro + 2 < OWN:
                        res_row(2 * ro + 2)
                    pool_row(ro)
                    resrows.pop(2 * ro - 1, None)
                    resrows.pop(2 * ro, None)

    nc.compile()
    _NC_CACHE = nc
    return nc


def _pad_plane(a):
    """[..., C, R, 160, 32] -> [..., C, R, PLANE] with d2/d3 pads."""
    C, R = a.shape[0], a.shape[1]
    out = np.zeros((C, R, D2P, D3P), a.dtype)
    out[:, :, 1:161, 1:33] = a
    return out.reshape(C, R, PLANE)


def _rows_slice(full, s):
    """full [C, 160, 160, 32] -> zero-padded rows s-2 .. s+20 (23 rows)."""
    C = full.shape[0]
    out = np.zeros((C, XROWS, D2, D3), full.dtype)
    lo, hi = s - 2, s + 21
    clo, chi = max(lo, 0), min(hi, D1)
    out[:, clo - lo:chi - lo] = full[:, clo:chi]
    return out


def prep_in_maps(x, mask, W_A1, W_A2, W_B1, W_B2, W_pool,
                 g_A1, b_A1, g_A2, b_A2, g_B1, b_B1, g_B2, b_B2):
    x = np.asarray(x)
    m = np.asarray(mask).astype(np.float32)

    # ---- weight layouts (fp16) ----
    def packW(Wt, cin, taps):
        # Wt[o, i, t0, t1] with the in-plane tap axis LAST (already selected)
        pass

    WA1 = np.asarray(W_A1)   # [64, 32, 3, 1, 3] taps (dd, dw)
    WB1 = np.asarray(W_B1)   # [64, 32, 1, 3, 3] taps (dh, dw)
    WA2 = np.asarray(W_A2)   # [64, 64, 1, 3, 3]
    WB2 = np.asarray(W_B2)   # [64, 64, 3, 1, 3]
    WP = np.asarray(W_pool)  # [64, 64, 3, 3, 3]

    wA1 = np.zeros((96, 3, 64), np.float16)
    wB1 = np.zeros((96, 3, 64), np.float16)
    for j in range(3):
        for i in range(CIN):
            wA1[j * 32 + i, :, :] = WA1[:, i, :, 0, j].T      # [dd] -> [3,64]
            wB1[j * 32 + i, :, :] = WB1[:, i, 0, :, j].T      # [dh]
    wA2f = np.zeros((128, 3, 64), np.float16)
    wA2h = np.zeros((64, 3, 64), np.float16)
    wB2f = np.zeros((128, 3, 64), np.float16)
    wB2h = np.zeros((64, 3, 64), np.float16)
    for i in range(COUT):
        for j in range(2):
            wA2f[j * 64 + i] = WA2[:, i, 0, :, j + 1].T
            wB2f[j * 64 + i] = WB2[:, i, :, 0, j + 1].T
        wA2h[i] = WA2[:, i, 0, :, 0].T
        wB2h[i] = WB2[:, i, :, 0, 0].T
    wPf = np.zeros((128, 9, 64), np.float16)
    wPh = np.zeros((64, 9, 64), np.float16)
    for i in range(COUT):
        for kd in range(3):
            for kh in range(3):
                for j in range(2):
                    wPf[j * 64 + i, kd * 3 + kh] = WP[:, i, kd, kh, j + 1].T
                wPh[i, kd * 3 + kh] = WP[:, i, kd, kh, 0]
    gbv = np.stack([np.asarray(v).astype(np.float32) for v in
                    (g_A1, b_A1, g_A2, b_A2, g_B1, b_B1, g_B2, b_B2)], 1)

    x4 = x[0]                    # [32, 160, 160, 32]
    m4 = m[0, 0]                 # [160, 160, 32]

    in_maps = []
    for k in range(NCORES):
        s = k * OWN
        xs = _rows_slice(x4, s).astype(np.float16)
        ms = _rows_slice(m4[None], s)[0]          # [23, 160, 32] f32
        ms64 = np.broadcast_to(ms[None], (COUT, XROWS, D2, D3)).astype(np.float16)
        mf = m4[s:s + OWN].astype(np.float32).reshape(128, 800)
        in_maps.append({
            "x_sh": np.ascontiguousarray(_pad_plane(xs)),
            "m_sh": np.ascontiguousarray(_pad_plane(ms64)),
            "mflat": np.ascontiguousarray(mf),
            "wA1": wA1, "wB1": wB1, "wA2f": wA2f, "wA2h": wA2h,
            "wB2f": wB2f, "wB2h": wB2h, "wPf": wPf, "wPh": wPh,
            "gb": np.ascontiguousarray(gbv),
        })

    return in_maps


def assemble(res_list):
    res_B = np.zeros((1, COUT, D1, D2, D3), np.float32)
    down = np.zeros((1, COUT, D1 // 2, D2 // 2, D3), np.float32)
    for k in range(NCORES):
        r = res_list[k]
        rp = r["res_out"].astype(np.float32).reshape(COUT, OWN, D2P, D3P)
        res_B[0, :, k * OWN:(k + 1) * OWN] = rp[:, :, 1:161, 1:33]
        dp = r["down_out"].reshape(COUT, PO_ROWS, 80, 32)
        down[0, :, k * PO_ROWS:(k + 1) * PO_ROWS] = dp
    return (down, res_B)


def kernel(**inputs):
    global LAST_RESULTS
    in_maps = prep_in_maps(**inputs)
    nc = build_program()
    results = run_bass_kernel_spmd(nc, in_maps, core_ids=list(range(NCORES)),
                                   trace=TRACE)
    LAST_RESULTS = results
    res_list = results.results if hasattr(results, "results") else results
    return assemble(res_list)


# revision 10
# speedup vs baseline: 27.0513x; 27.0513x over previous
import sys
sys.path.insert(0, '/opt/trn_rl_repo')
import numpy as np
import concourse.bass as bass
import concourse.bacc as bacc
import concourse.tile as tile
import concourse.mybir as mybir
import bass_rust
from concourse.bass_utils import run_bass_kernel_spmd

F32 = mybir.dt.float32
F16 = mybir.dt.float16
AF = mybir.ActivationFunctionType
ALU = mybir.AluOpType

NCORES = 8
CIN, COUT = 32, 64
D1, D2, D3 = 160, 160, 32
OWN = D1 // NCORES          # 20 owned d1-rows per core
D2P, D3P = D2 + 2, D3 + 1   # padded plane: 162 x 33 (d3 pad col shared)
PLANE = D2P * D3P           # 5346
G = 34                      # tile guard columns each side (max |shift| = 34)
W = PLANE + 2 * G + 2       # sbuf row-tile width
NSPLIT = 15 * D3P           # matmul chunk: 15 d2-rows = 495 positions
EPS = 1e-5
SLOPE = 0.01

# pool output geometry: 10 out rows/core, positions (do2 in [0,80), do3 in [0,32))
PO_ROWS = OWN // 2
PO_N = 80 * 32              # 2560, chunked by 16 do2-rows = 512
PO_CHUNK = 16 * 32

XROWS = 23   # x / mask rows per core: logical d1 = own_start-2 .. own_start+20
ZA1_ROWS = 21  # rows -1..19 (slot = r+1)
ZB1_ROWS = 23  # rows -2..20 (slot = r+2)
Z2_ROWS = 21   # zA2/zB2 rows -1..19 (slot = r+1)


def _win(ap, dims):
    """Overlapping multi-dim window view: keep ap's partition dim + offset,
    replace free dims with [[step, count], ...] (element units)."""
    c = ap.copy()
    part = [list(p) for p in c.ap][0]
    c.ap = bass_rust.VecI64Pair([part] + [list(d) for d in dims])
    return c


def _chunks():
    """(start, size) chunks of the plane, d2-aligned, size<=512."""
    out = []
    s = 0
    while s < PLANE:
        n = min(NSPLIT, PLANE - s)
        out.append((s, n))
        s += n
    return out


CHUNKS = _chunks()  # 10x495 + 396

TRACE = False          # set by test.py to capture an NTFF profile
LAST_RESULTS = None
REPEAT = 1             # emit the whole compute body N times (for timing)
_NC_CACHE = {}


def build_program():
    if REPEAT in _NC_CACHE:
        return _NC_CACHE[REPEAT]
    nc = bacc.Bacc("TRN2", target_bir_lowering=False, debug=False,
                   num_devices=NCORES)

    # ---- external I/O (per-core shards) ----
    x_sh = nc.dram_tensor("x_sh", [CIN, XROWS, PLANE], F16, kind="ExternalInput")
    m_sh = nc.dram_tensor("m_sh", [COUT, XROWS, PLANE], F16, kind="ExternalInput")
    mflat = nc.dram_tensor("mflat", [128, 800], F32, kind="ExternalInput")
    wA1 = nc.dram_tensor("wA1", [96, 3, 64], F16, kind="ExternalInput")
    wB1 = nc.dram_tensor("wB1", [96, 3, 64], F16, kind="ExternalInput")
    wA2f = nc.dram_tensor("wA2f", [128, 3, 64], F16, kind="ExternalInput")
    wA2h = nc.dram_tensor("wA2h", [64, 3, 64], F16, kind="ExternalInput")
    wB2f = nc.dram_tensor("wB2f", [128, 3, 64], F16, kind="ExternalInput")
    wB2h = nc.dram_tensor("wB2h", [64, 3, 64], F16, kind="ExternalInput")
    wPf = nc.dram_tensor("wPf", [128, 9, 64], F16, kind="ExternalInput")
    wPh = nc.dram_tensor("wPh", [64, 9, 64], F16, kind="ExternalInput")
    gb = nc.dram_tensor("gb", [64, 8], F32, kind="ExternalInput")
    res_out = nc.dram_tensor("res_out", [COUT, OWN, PLANE], F16,
                             kind="ExternalOutput")
    down_out = nc.dram_tensor("down_out", [COUT, PO_ROWS, PO_N], F32,
                              kind="ExternalOutput")

    def emit(tc):
        with (
            tc.tile_pool(name="wpool", bufs=1) as wpool,
            tc.tile_pool(name="stats", bufs=1) as spool,
            tc.tile_pool(name="dram", bufs=1, space="DRAM") as dram,
            tc.tile_pool(name="psum_conv", bufs=2, space="PSUM") as pconv,
            tc.tile_pool(name="psum_aux", bufs=2, space="PSUM") as paux,
        ):
            # ---------- persistent weights ----------
            wA1t = wpool.tile([96, 3, 64], F16)
            wB1t = wpool.tile([96, 3, 64], F16)
            wA2ft = wpool.tile([128, 3, 64], F16)
            wA2ht = wpool.tile([64, 3, 64], F16)
            wB2ft = wpool.tile([128, 3, 64], F16)
            wB2ht = wpool.tile([64, 3, 64], F16)
            wPft = wpool.tile([128, 9, 64], F16)
            wPht = wpool.tile([64, 9, 64], F16)
            # BN-scaled copies for pass 2
            wA2fs = wpool.tile([128, 3, 64], F16)
            wA2hs = wpool.tile([64, 3, 64], F16)
            wB2fs = wpool.tile([128, 3, 64], F16)
            wB2hs = wpool.tile([64, 3, 64], F16)
            gbt = wpool.tile([64, 8], F32)
            for t, d in ((wA1t, wA1), (wB1t, wB1), (wA2ft, wA2f), (wA2ht, wA2h),
                         (wB2ft, wB2f), (wB2ht, wB2h), (wPft, wPf), (wPht, wPh),
                         (gbt, gb)):
                nc.sync.dma_start(t[:], d[:])

            # ---------- n_active (replicated on 64 partitions) ----------
            mft = spool.tile([128, 800], F32)
            nc.sync.dma_start(mft[:], mflat[:])
            mred = spool.tile([128, 1], F32)
            nc.vector.tensor_reduce(mred[:], mft[:], axis=mybir.AxisListType.X,
                                    op=ALU.add)
            ones128 = spool.tile([128, 64], F16)
            nc.vector.memset(ones128[:], 1.0)
            mred16 = spool.tile([128, 1], F16)
            nc.vector.tensor_copy(mred16[:], mred[:])
            nps = paux.tile([64, 1], F32)
            nc.tensor.matmul(nps[:], ones128[:], mred16[:], start=True, stop=True)
            nvec = spool.tile([64, 1], F32)
            nc.vector.tensor_copy(nvec[:], nps[:])

            # dram intermediates
            zA1d = dram.tile([COUT, ZA1_ROWS, PLANE], F16)
            zB1d = dram.tile([COUT, ZB1_ROWS, PLANE], F16)
            zA2d = dram.tile([COUT, Z2_ROWS, PLANE], F16)
            zB2d = dram.tile([COUT, Z2_ROWS, PLANE], F16)

            # stats accumulators (per conv layer): per-row (mean, var)
            rowagg = {k: spool.tile([64, OWN, 2], F32, name=f"rowagg_{k}")
                      for k in ("A1", "B1", "A2", "B2")}

            # ======================================================
            # PASS 1:  A1 = conv(xs, W_A1 (3,1,3)),  B1 = conv(xs, W_B1 (1,3,3))
            # ======================================================
            with (
                tc.tile_pool(name="pk1", bufs=4) as pk1pool,
                tc.tile_pool(name="mrow1", bufs=4) as mpool1,
                tc.tile_pool(name="zst1", bufs=2) as zst1,
                tc.tile_pool(name="tmp1", bufs=3) as tmp1,
                tc.tile_pool(name="bst1", bufs=2) as bst1,
            ):
                packs = {}   # xr -> pack tile [96, W]
                mrows = {}   # xr -> mask row tile [64, W]

                def load_row_p1(xr):
                    pk = pk1pool.tile([96, W], F16, tag="pk")
                    mr = mpool1.tile([64, W], F16, tag="mr")
                    if xr < 4:
                        nc.vector.memset(pk[:], 0.0)
                        nc.vector.memset(mr[:], 0.0)
                    nc.sync.dma_start(mr[:, G:G + PLANE], m_sh[:, xr, :])
                    nc.sync.dma_start(pk[32:64, G:G + PLANE], x_sh[:, xr, :])
                    # mask in place:  xs = x * m
                    nc.vector.tensor_tensor(
                        out=pk[32:64, G:G + PLANE], in0=pk[32:64, G:G + PLANE],
                        in1=mr[32:64, G:G + PLANE], op=ALU.mult)
                    # shifted replicas: block0 = xs(pos-1), block2 = xs(pos+1)
                    nc.sync.dma_start(pk[0:32, 1:W], pk[32:64, 0:W - 1])
                    nc.sync.dma_start(pk[64:96, 0:W - 1], pk[32:64, 1:W])
                    packs[xr] = pk
                    mrows[xr] = mr

                def conv_row(kind, r):
                    # kind 'A1': out-row r, taps (dd, dw): packs r-1,r,r+1
                    # kind 'B1': out-row r, taps (dh, dw): pack r only
                    zrow = zst1.tile([64, PLANE], F16, tag=f"z{kind}")
                    own = 0 <= r < OWN
                    if own:
                        bst = bst1.tile([64, len(CHUNKS), 6], F32, tag=f"b{kind}")
                    mr = mrows[r + 2]
                    for ci, (cs, cn) in enumerate(CHUNKS):
                        ps = pconv.tile([64, NSPLIT], F32, tag="conv")
                        for k in range(3):
                            if kind == "A1":
                                pk = packs[r + 1 + k]
                                rhs = pk[0:96, G + cs:G + cs + cn]
                                lhsT = wA1t[:, k, :]
                            else:
                                pk = packs[r + 2]
                                off = (k - 1) * D3P
                                rhs = pk[0:96, G + cs + off:G + cs + off + cn]
                                lhsT = wB1t[:, k, :]
                            nc.tensor.matmul(ps[:, 0:cn], lhsT, rhs,
                                             start=(k == 0), stop=(k == 2))
                        tchunk = tmp1.tile([64, NSPLIT], F16, tag="t")
                        nc.scalar.activation(tchunk[:, 0:cn], ps[:, 0:cn],
                                             AF.Lrelu, alpha=SLOPE)
                        nc.vector.tensor_tensor(
                            out=zrow[:, cs:cs + cn], in0=tchunk[:, 0:cn],
                            in1=mr[:, G + cs:G + cs + cn], op=ALU.mult)
                        if own:
                            nc.vector.bn_stats(bst[:, ci, :], zrow[:, cs:cs + cn])
                    if own:
                        nc.vector.bn_aggr(rowagg[kind][:, r, :], bst[:])
                    dst = zA1d if kind == "A1" else zB1d
                    slot = r + 1 if kind == "A1" else r + 2
                    nc.sync.dma_start(dst[:, slot, :], zrow[:])

                for xr in range(XROWS):
                    load_row_p1(xr)
                    rl = xr - 2           # logical d1 row just loaded
                    if -2 <= rl <= 20:
                        conv_row("B1", rl)
                    ra = rl - 1
                    if -1 <= ra < 20:
                        conv_row("A1", ra)
                    # free old pack/mask refs (pool rotation handles reuse)
                    packs.pop(xr - 3, None)
                    mrows.pop(xr - 3, None)

            # ======================================================
            # stats -> allreduce #1 -> BN affine params for A1, B1
            # ======================================================
            def finalize_stats(keys, tag):
                st = spool.tile([64, 5], F32, name=f"stats_{tag}")
                for i, k in enumerate(keys):
                    ra = rowagg[k]
                    t1 = spool.tile([64, OWN], F32, name=f"t1_{k}")
                    nc.vector.tensor_tensor(out=t1[:], in0=ra[:, :, 0],
                                            in1=ra[:, :, 0], op=ALU.mult)
                    nc.vector.tensor_tensor(out=t1[:], in0=t1[:],
                                            in1=ra[:, :, 1], op=ALU.add)
                    # sum z = PLANE * sum(mean_r);  sum z^2 = PLANE * sum(var+mean^2)
                    s0 = spool.tile([64, 1], F32, name=f"s0_{k}")
                    nc.vector.tensor_reduce(s0[:], ra[:, :, 0],
                                            axis=mybir.AxisListType.X, op=ALU.add)
                    nc.vector.tensor_scalar_mul(st[:, 2 * i:2 * i + 1], s0[:],
                                                float(PLANE))
                    s1 = spool.tile([64, 1], F32, name=f"s1_{k}")
                    nc.vector.tensor_reduce(s1[:], t1[:],
                                            axis=mybir.AxisListType.X, op=ALU.add)
                    nc.vector.tensor_scalar_mul(st[:, 2 * i + 1:2 * i + 2], s1[:],
                                                float(PLANE))
                nc.vector.tensor_copy(st[:, 4:5], nvec[:])
                bin_ = dram.tile([64, 5], F32, name=f"arin_{tag}")
                bout = dram.tile([64, 5], F32, name=f"arout_{tag}")
                nc.sync.dma_start(bin_[:], st[:])
                nc.gpsimd.collective_compute(
                    "AllReduce", ALU.add,
                    replica_groups=[list(range(NCORES))],
                    ins=[bin_.opt()], outs=[bout.opt()])
                stg = spool.tile([64, 5], F32, name=f"arres_{tag}")
                nc.sync.dma_start(stg[:], bout[:])
                return stg

            def bn_params(stg, i, gcol, bcol, tag):
                # returns (a [64,1] f32, b [64,1] f32)
                rn = spool.tile([64, 1], F32, name=f"rn_{tag}")
                nc.vector.reciprocal(rn[:], stg[:, 4:5])
                mu = spool.tile([64, 1], F32, name=f"mu_{tag}")
                nc.vector.tensor_tensor(out=mu[:], in0=stg[:, 2 * i:2 * i + 1],
                                        in1=rn[:], op=ALU.mult)
                ez2 = spool.tile([64, 1], F32, name=f"ez2_{tag}")
                nc.vector.tensor_tensor(out=ez2[:], in0=stg[:, 2 * i + 1:2 * i + 2],
                                        in1=rn[:], op=ALU.mult)
                var = spool.tile([64, 1], F32, name=f"var_{tag}")
                nc.vector.tensor_tensor(out=var[:], in0=mu[:], in1=mu[:],
                                        op=ALU.mult)
                nc.vector.tensor_tensor(out=var[:], in0=ez2[:], in1=var[:],
                                        op=ALU.subtract)
                nc.vector.tensor_scalar_add(var[:], var[:], EPS)
                sd = spool.tile([64, 1], F32, name=f"sd_{tag}")
                nc.scalar.activation(sd[:], var[:], AF.Sqrt)
                inv = spool.tile([64, 1], F32, name=f"inv_{tag}")
                nc.vector.reciprocal(inv[:], sd[:])
                a = spool.tile([64, 1], F32, name=f"a_{tag}")
                nc.vector.tensor_tensor(out=a[:], in0=inv[:],
                                        in1=gbt[:, gcol:gcol + 1], op=ALU.mult)
                b = spool.tile([64, 1], F32, name=f"b_{tag}")
                nc.vector.tensor_tensor(out=b[:], in0=mu[:], in1=a[:],
                                        op=ALU.mult)
                nc.vector.tensor_tensor(out=b[:], in0=gbt[:, bcol:bcol + 1],
                                        in1=b[:], op=ALU.subtract)
                return a, b

            def scale_weights(a, full_raw, full_s, half_raw, half_s, tag):
                a128 = spool.tile([128, 1], F32, name=f"a128_{tag}")
                nc.vector.tensor_copy(a128[0:64, :], a[:])
                nc.sync.dma_start(a128[64:128, :], a[:])
                nc.vector.tensor_scalar_mul(
                    full_s[:].rearrange("p a b -> p (a b)"),
                    full_raw[:].rearrange("p a b -> p (a b)"), a128[:])
                nc.vector.tensor_scalar_mul(
                    half_s[:].rearrange("p a b -> p (a b)"),
                    half_raw[:].rearrange("p a b -> p (a b)"), a[:])

            def kappa(b, full_raw, half_raw, order, tag, base=0):
                # kappa[t,o] = sum_i W_raw[o,i,tap]*b_i, laid out [9,64] rows=t
                b16 = spool.tile([128, 1], F16, name=f"b16_{tag}")
                nc.vector.tensor_copy(b16[0:64, :], b[:])
                nc.sync.dma_start(b16[64:128, :], b16[0:64, :])
                kT = spool.tile([64, 32], F32, name=f"kT_{tag}")
                nc.vector.memset(kT[:], 0.0)
                for t, (blk, idx) in enumerate(order):
                    src = full_raw if blk >= 0 else half_raw
                    if blk >= 0:
                        lhsT = src[64 * blk:64 * blk + 64, idx, :]
                        rhs = b16[64 * blk:64 * blk + 64, :]
                    else:
                        lhsT, rhs = src[:, idx, :], b16[0:64, :]
                    kp = paux.tile([64, 1], F32, tag="tiny")
                    nc.tensor.matmul(kp[:], lhsT, rhs, start=True, stop=True)
                    nc.scalar.copy(kT[:, t:t + 1], kp[:])
                kTt = spool.tile([64, 64], F32, name=f"kTt_{tag}")
                nc.vector.transpose(kTt[0:32, 0:32], kT[0:32, :])
                nc.vector.transpose(kTt[0:32, 32:64], kT[32:64, :])
                ka32 = spool.tile([32, 64], F16, name=f"ka32_{tag}")
                nc.vector.tensor_copy(ka32[:], kTt[0:32, :])
                if base == 0:
                    return ka32
                ka = spool.tile([64, 64], F16, name=f"ka_{tag}")
                nc.sync.dma_start(ka[base:base + 9, :], ka32[0:9, :])
                return ka

            stg1 = finalize_stats(("A1", "B1"), "ar1")
            aA1, bA1 = bn_params(stg1, 0, 0, 1, "A1")
            aB1, bB1 = bn_params(stg1, 1, 4, 5, "B1")
            scale_weights(aA1, wA2ft, wA2fs, wA2ht, wA2hs, "A2")
            scale_weights(aB1, wB2ft, wB2fs, wB2ht, wB2hs, "B2")
            # kappa col order must match mp row order:
            # A2 rows: t = kw*3+dh  -> tap (dh, kw)
            ordA2 = [(kw - 1 if kw >= 1 else -1, dh)
                     for kw in range(3) for dh in range(3)]
            # B2 rows: t = dd*3+kw -> tap (dd, kw)
            ordB2 = [(kw - 1 if kw >= 1 else -1, dd)
                     for dd in range(3) for kw in range(3)]
            kaA2 = kappa(bA1, wA2ft, wA2ht, ordA2, "A2")
            kaB2 = kappa(bB1, wB2ft, wB2ht, ordB2, "B2", base=32)

            # ======================================================
            # PASS 2:  A2 = conv(u_A1, (1,3,3)),  B2 = conv(u_B1, (3,1,3))
            #   u = a*z + b*m  folded as: scaled weights + kappa-bias matmul
            # ======================================================
            with (
                tc.tile_pool(name="pkA2", bufs=2) as pkA2pool,
                tc.tile_pool(name="pkB2", bufs=4) as pkB2pool,
                tc.tile_pool(name="mrow2", bufs=4) as mpool2,
                tc.tile_pool(name="mp2", bufs=2) as mppool,
                tc.tile_pool(name="zst2", bufs=2) as zst2,
                tc.tile_pool(name="tmp2", bufs=3) as tmp2,
                tc.tile_pool(name="bst2", bufs=2) as bst2,
            ):
                pkB = {}
                mrows2 = {}

                def load_packA2(r):
                    pk = pkA2pool.tile([128, W], F16, tag="pkA")
                    if r < 1:
                        nc.vector.memset(pk[:], 0.0)
                    nc.sync.dma_start(pk[0:64, G:G + PLANE], zA1d[:, r + 1, :])
                    nc.sync.dma_start(pk[64:128, 0:W - 1], pk[0:64, 1:W])
                    return pk

                def load_packB2(rz):
                    pk = pkB2pool.tile([128, W], F16, tag="pkB")
                    if rz < 2:
                        nc.vector.memset(pk[:], 0.0)
                    nc.sync.dma_start(pk[0:64, G:G + PLANE], zB1d[:, rz + 2, :])
                    nc.sync.dma_start(pk[64:128, 0:W - 1], pk[0:64, 1:W])
                    pkB[rz] = pk

                def load_mask2(r):
                    mr = mpool2.tile([64, W], F16, tag="mr2")
                    if r < 2:
                        nc.vector.memset(mr[:], 0.0)
                    nc.sync.dma_start(mr[:, G:G + PLANE], m_sh[:, r + 2, :])
                    mrows2[r] = mr

                def build_mp(r):
                    # rows 0:9   A2 windows of mask row r: t=kw*3+dh
                    # rows 32:41 B2 windows rows r-1..r+1: t=dd*3+kw
                    # rows 64:67 scratch strip: kw-windows of row r
                    mp = mppool.tile([67, W], F16, tag="mp")
                    mr = mrows2[r]
                    nc.sync.dma_start(
                        mp[64:67, 1:W - 2],
                        _win(mr[0:1, 0:W - 3], [[1, 3], [1, W - 3]]))
                    nc.sync.dma_start(
                        mp[0:9, G:G + PLANE],
                        _win(mp[64:67, G - D3P:G - D3P + PLANE],
                             [[D3P, 3], [1, PLANE]]))
                    for dd in range(3):
                        src = mrows2[r + dd - 1]
                        nc.sync.dma_start(
                            mp[32 + 3 * dd:35 + 3 * dd, G:G + PLANE],
                            _win(src[0:1, G - 1:G - 1 + PLANE],
                                 [[1, 3], [1, PLANE]]))
                    return mp

                def conv_row2(kind, r, pkA, mp):
                    zrow = zst2.tile([64, PLANE], F16, tag=f"z{kind}")
                    own = 0 <= r < OWN
                    if own:
                        bst = bst2.tile([64, len(CHUNKS), 6], F32, tag=f"b{kind}")
                    mr = mrows2[r]
                    for ci, (cs, cn) in enumerate(CHUNKS):
                        ps = pconv.tile([64, NSPLIT], F32, tag="conv")
                        nmm = 0
                        for k in range(3):
                            if kind == "A2":
                                pk, off = pkA, (k - 1) * D3P
                                wf, wh = wA2fs, wA2hs
                            else:
                                pk, off = pkB[r + k - 1], 0
                                wf, wh = wB2fs, wB2hs
                            # full: blocks (j=0,1) = taps kw=1,2 at offset 0
                            rhs = pk[0:128, G + cs + off:G + cs + off + cn]
                            nc.tensor.matmul(ps[:, 0:cn], wf[:, k, :], rhs,
                                             start=(nmm == 0), stop=False)
                            nmm += 1
                            rhs = pk[0:64, G + cs + off - 1:G + cs + off - 1 + cn]
                            nc.tensor.matmul(ps[:, 0:cn], wh[:, k, :], rhs,
                                             start=False, stop=False)
                            nmm += 1
                        if kind == "A2":
                            ka, mpr = kaA2[0:9, :], mp[0:9, G + cs:G + cs + cn]
                        else:
                            ka, mpr = kaB2[32:41, :], mp[32:41, G + cs:G + cs + cn]
                        nc.tensor.matmul(ps[:, 0:cn], ka, mpr,
                                         start=False, stop=True)
                        tchunk = tmp2.tile([64, NSPLIT], F16, tag="t")
                        nc.scalar.activation(tchunk[:, 0:cn], ps[:, 0:cn],
                                             AF.Lrelu, alpha=SLOPE)
                        nc.vector.tensor_tensor(
                            out=zrow[:, cs:cs + cn], in0=tchunk[:, 0:cn],
                            in1=mr[:, G + cs:G + cs + cn], op=ALU.mult)
                        if own:
                            nc.vector.bn_stats(bst[:, ci, :], zrow[:, cs:cs + cn])
                    if own:
                        nc.vector.bn_aggr(rowagg[kind][:, r, :], bst[:])
                    dst = zA2d if kind == "A2" else zB2d
                    nc.sync.dma_start(dst[:, r + 1, :], zrow[:])

                # prologue loads: zB1 rows rz=-2,-1 ; mask rows -2..-1
                for rz in (-2, -1):
                    load_packB2(rz)
                load_mask2(-2)
                load_mask2(-1)
                for r in range(-1, OWN):
                    load_packB2(r + 1)
                    load_mask2(r + 1)
                    pkA = load_packA2(r)
                    mp = build_mp(r)
                    conv_row2("A2", r, pkA, mp)
                    conv_row2("B2", r, pkA, mp)
                    pkB.pop(r - 1, None)
                    mrows2.pop(r - 1, None)

            # ======================================================
            # stats -> allreduce #2 -> res_B materialization + pool conv
            # ======================================================
            stg2 = finalize_stats(("A2", "B2"), "ar2")
            aA2, bA2 = bn_params(stg2, 0, 2, 3, "A2f")
            aB2, bB2 = bn_params(stg2, 1, 6, 7, "B2f")
            # bAB row [1,64] for the rank-1 mask bias matmul
            bAB = spool.tile([64, 1], F32, name="bAB")
            nc.vector.tensor_tensor(out=bAB[:], in0=bA2[:], in1=bB2[:], op=ALU.add)
            bABsq = spool.tile([64, 32], F32, name="bABsq")
            nc.vector.memset(bABsq[:], 0.0)
            nc.vector.tensor_copy(bABsq[:, 0:1], bAB[:])
            bABt = spool.tile([64, 64], F32, name="bABt")
            nc.vector.transpose(bABt[0:32, 0:32], bABsq[0:32, :])
            nc.vector.transpose(bABt[0:32, 32:64], bABsq[32:64, :])
            bABrow = spool.tile([1, 64], F16, name="bABrow")
            nc.vector.tensor_copy(bABrow[:], bABt[0:1, :])

            with (
                tc.tile_pool(name="z3", bufs=2) as z3pool,
                tc.tile_pool(name="res3", bufs=4) as res3pool,
                tc.tile_pool(name="mrow3", bufs=2) as mpool3,
                tc.tile_pool(name="dst3", bufs=2) as dst3,
            ):
                resrows = {}

                def res_row(r):
                    # res = aA2*zA2 + aB2*zB2 + bAB*m   (rows -1..19)
                    zta = z3pool.tile([64, PLANE], F16, tag="z3a")
                    ztb = z3pool.tile([64, PLANE], F16, tag="z3b")
                    nc.sync.dma_start(zta[:], zA2d[:, r + 1, :])
                    nc.sync.dma_start(ztb[:], zB2d[:, r + 1, :])
                    mr = mpool3.tile([64, W], F16, tag="mr3")
                    if r < 1:
                        nc.vector.memset(mr[:], 0.0)
                    nc.sync.dma_start(mr[:, G:G + PLANE], m_sh[:, r + 2, :])
                    pk = res3pool.tile([128, W], F16, tag="res")
                    if r < 3:
                        nc.vector.memset(pk[:], 0.0)
                    for ci, (cs, cn) in enumerate(CHUNKS):
                        cm = paux.tile([64, NSPLIT], F32, tag="cm")
                        nc.tensor.matmul(cm[:, 0:cn], bABrow[:],
                                         mr[0:1, G + cs:G + cs + cn],
                                         start=True, stop=True)
                        t3 = dst3.tile([64, NSPLIT], F32, tag="t3")
                        nc.vector.scalar_tensor_tensor(
                            out=t3[:, 0:cn], in0=zta[:, cs:cs + cn],
                            scalar=aA2[:], in1=cm[:, 0:cn],
                            op0=ALU.mult, op1=ALU.add)
                        nc.vector.scalar_tensor_tensor(
                            out=pk[0:64, G + cs:G + cs + cn],
                            in0=ztb[:, cs:cs + cn],
                            scalar=aB2[:], in1=t3[:, 0:cn],
                            op0=ALU.mult, op1=ALU.add)
                    # replicate for pool pack: block1 = res(pos+1)
                    nc.sync.dma_start(pk[64:128, 0:W - 1], pk[0:64, 1:W])
                    if 0 <= r < OWN:
                        nc.sync.dma_start(res_out[:, r, :],
                                          pk[0:64, G:G + PLANE])
                    resrows[r] = pk

                def pool_row(ro):
                    # down rows: reads res rows 2ro-1, 2ro, 2ro+1
                    for c in range(5):
                        ps = pconv.tile([64, PO_CHUNK], F32, tag="conv")
                        base = c * 16 * 2 * D3P
                        nmm = 0
                        for kd in range(3):
                            pk = resrows[2 * ro + kd - 1]
                            for kh in range(3):
                                off = G + base + kh * D3P
                                rhs = _win(pk[0:128, off + 1:off + 1 + PO_CHUNK],
                                           [[2 * D3P, 16], [1, 32]])
                                nc.tensor.matmul(
                                    ps[:], wPft[:, 3 * kd + kh, :], rhs,
                                    start=(nmm == 0), stop=False)
                                nmm += 1
                                rhs = _win(pk[0:64, off:off + PO_CHUNK],
                                           [[2 * D3P, 16], [1, 32]])
                                last = (kd == 2 and kh == 2)
                                nc.tensor.matmul(
                                    ps[:], wPht[:, 3 * kd + kh, :], rhs,
                                    start=False, stop=last)
                                nmm += 1
                        dc = dst3.tile([64, PO_CHUNK], F32, tag="dchunk")
                        nc.vector.tensor_copy(dc[:], ps[:])
                        nc.sync.dma_start(
                            down_out[:, ro, c * PO_CHUNK:(c + 1) * PO_CHUNK],
                            dc[:])

                res_row(-1)
                res_row(0)
                for ro in range(PO_ROWS):
                    res_row(2 * ro + 1)
                    if 2 * ro + 2 < OWN:
                        res_row(2 * ro + 2)
                    pool_row(ro)
                    resrows.pop(2 * ro - 1, None)
                    resrows.pop(2 * ro, None)

    with tile.TileContext(nc) as tc:
        for _rep in range(REPEAT):
            emit(tc)
    nc.compile()
    _NC_CACHE[REPEAT] = nc
    return nc


def _pad_plane(a):
    """[..., C, R, 160, 32] -> [..., C, R, PLANE] with d2/d3 pads."""
    C, R = a.shape[0], a.shape[1]
    out = np.zeros((C, R, D2P, D3P), a.dtype)
    out[:, :, 1:161, 1:33] = a
    return out.reshape(C, R, PLANE)


def _rows_slice(full, s):
    """full [C, 160, 160, 32] -> zero-padded rows s-2 .. s+20 (23 rows)."""
    C = full.shape[0]
    out = np.zeros((C, XROWS, D2, D3), full.dtype)
    lo, hi = s - 2, s + 21
    clo, chi = max(lo, 0), min(hi, D1)
    out[:, clo - lo:chi - lo] = full[:, clo:chi]
    return out


def prep_in_maps(x, mask, W_A1, W_A2, W_B1, W_B2, W_pool,
                 g_A1, b_A1, g_A2, b_A2, g_B1, b_B1, g_B2, b_B2):
    x = np.asarray(x)
    m = np.asarray(mask).astype(np.float32)

    # ---- weight layouts (fp16) ----
    def packW(Wt, cin, taps):
        # Wt[o, i, t0, t1] with the in-plane tap axis LAST (already selected)
        pass

    WA1 = np.asarray(W_A1)   # [64, 32, 3, 1, 3] taps (dd, dw)
    WB1 = np.asarray(W_B1)   # [64, 32, 1, 3, 3] taps (dh, dw)
    WA2 = np.asarray(W_A2)   # [64, 64, 1, 3, 3]
    WB2 = np.asarray(W_B2)   # [64, 64, 3, 1, 3]
    WP = np.asarray(W_pool)  # [64, 64, 3, 3, 3]

    wA1 = np.zeros((96, 3, 64), np.float16)
    wB1 = np.zeros((96, 3, 64), np.float16)
    for j in range(3):
        for i in range(CIN):
            wA1[j * 32 + i, :, :] = WA1[:, i, :, 0, j].T      # [dd] -> [3,64]
            wB1[j * 32 + i, :, :] = WB1[:, i, 0, :, j].T      # [dh]
    wA2f = np.zeros((128, 3, 64), np.float16)
    wA2h = np.zeros((64, 3, 64), np.float16)
    wB2f = np.zeros((128, 3, 64), np.float16)
    wB2h = np.zeros((64, 3, 64), np.float16)
    for i in range(COUT):
        for j in range(2):
            wA2f[j * 64 + i] = WA2[:, i, 0, :, j + 1].T
            wB2f[j * 64 + i] = WB2[:, i, :, 0, j + 1].T
        wA2h[i] = WA2[:, i, 0, :, 0].T
        wB2h[i] = WB2[:, i, :, 0, 0].T
    wPf = np.zeros((128, 9, 64), np.float16)
    wPh = np.zeros((64, 9, 64), np.float16)
    for i in range(COUT):
        for kd in range(3):
            for kh in range(3):
                for j in range(2):
                    wPf[j * 64 + i, kd * 3 + kh] = WP[:, i, kd, kh, j + 1].T
                wPh[i, kd * 3 + kh] = WP[:, i, kd, kh, 0]
    gbv = np.stack([np.asarray(v).astype(np.float32) for v in
                    (g_A1, b_A1, g_A2, b_A2, g_B1, b_B1, g_B2, b_B2)], 1)

    x4 = x[0]                    # [32, 160, 160, 32]
    m4 = m[0, 0]                 # [160, 160, 32]

    in_maps = []
    for k in range(NCORES):
        s = k * OWN
        xs = _rows_slice(x4, s).astype(np.float16)
        ms = _rows_slice(m4[None], s)[0]          # [23, 160, 32] f32
        ms64 = np.broadcast_to(ms[None], (COUT, XROWS, D2, D3)).astype(np.float16)
        mf = m4[s:s + OWN].astype(np.float32).reshape(128, 800)
        in_maps.append({
            "x_sh": np.ascontiguousarray(_pad_plane(xs)),
            "m_sh": np.ascontiguousarray(_pad_plane(ms64)),
            "mflat": np.ascontiguousarray(mf),
            "wA1": wA1, "wB1": wB1, "wA2f": wA2f, "wA2h": wA2h,
            "wB2f": wB2f, "wB2h": wB2h, "wPf": wPf, "wPh": wPh,
            "gb": np.ascontiguousarray(gbv),
        })

    return in_maps


def assemble(res_list):
    res_B = np.zeros((1, COUT, D1, D2, D3), np.float32)
    down = np.zeros((1, COUT, D1 // 2, D2 // 2, D3), np.float32)
    for k in range(NCORES):
        r = res_list[k]
        rp = r["res_out"].astype(np.float32).reshape(COUT, OWN, D2P, D3P)
        res_B[0, :, k * OWN:(k + 1) * OWN] = rp[:, :, 1:161, 1:33]
        dp = r["down_out"].reshape(COUT, PO_ROWS, 80, 32)
        down[0, :, k * PO_ROWS:(k + 1) * PO_ROWS] = dp
    return (down, res_B)


def kernel(**inputs):
    global LAST_RESULTS
    in_maps = prep_in_maps(**inputs)
    nc = build_program()
    results = run_bass_kernel_spmd(nc, in_maps, core_ids=list(range(NCORES)),
                                   trace=TRACE)
    LAST_RESULTS = results
    res_list = results.results if hasattr(results, "results") else results
    return assemble(res_list)
